# revision 38
# baseline (speedup 1.0000x reference)
"""Causal multi-head attention on 8 TRN2 NeuronCores.

Sharding: data-parallel over batch (2) x tensor-parallel over heads (4 groups
of 4 heads). Core c handles batch c//4, heads [4*(c%4), 4*(c%4)+4).
Each core computes Q/K/V projections for its head slice, causal flash-style
attention, and a partial output projection (Wo row-shard). The host sums the
4 partials per batch and adds bo.

Matmuls run in bf16 (fp32 accumulation in PSUM); X and the weight slices are
pre-cast to bf16 on the host so X^T comes from XBAR DMA transposes instead of
PE transposes. Softmax runs unnormalized (scores are ~N(0,1), no max
subtraction needed); the per-row sums ride along as a 65th column of V.

Shapes (per core): X [2048, 1024], WQ/WK/WV [1024, 256], WO [256, 1024].
"""

import ml_dtypes
import numpy as np

import concourse.bass as bass
import concourse.mybir as mybir
import concourse.tile as tile
from concourse import bacc
from concourse.bass_utils import run_bass_kernel_spmd

B = 2
S = 2048
D = 1024
H_PER_CORE = 4  # heads per core
HD = 64  # head dim
HG = H_PER_CORE * HD  # 256: projection slice width per core
P = 128
NQC = 4  # q chunks of 512
QC = S // NQC  # 512
NKB = S // P  # 16 k-blocks of 128
NEG = -1.0e9

f32 = mybir.dt.float32
bf16 = mybir.dt.bfloat16


def build_nc():
    nc = bacc.Bacc()

    XTD = nc.dram_tensor("XT", [D, S], bf16, kind="ExternalInput")
    WQ = nc.dram_tensor("WQ", [D, HG], bf16, kind="ExternalInput")
    WK = nc.dram_tensor("WK", [D, HG], bf16, kind="ExternalInput")
    WV = nc.dram_tensor("WV", [D, HG], bf16, kind="ExternalInput")
    WO = nc.dram_tensor("WO", [HG, D], bf16, kind="ExternalInput")
    BQ = nc.dram_tensor("BQ", [HG], f32, kind="ExternalInput")
    BK = nc.dram_tensor("BK", [HG], f32, kind="ExternalInput")
    BV = nc.dram_tensor("BV", [HG], f32, kind="ExternalInput")
    Y = nc.dram_tensor("Y", [S, D], bf16, kind="ExternalOutput")

    Y_pt = Y.rearrange("(t p) d -> p t d", p=P)

    with tile.TileContext(nc) as tc:
        with (
            tc.tile_pool(name="persist", bufs=1) as persist,
            tc.tile_pool(name="sb", bufs=2) as sb,
            tc.tile_pool(name="ps", bufs=1, space="PSUM") as ps,
        ):
            # ---- weights needed first (Q/K proj of chunk 0), split into
            # chunks so no single DMA queue serializes a whole tensor ----
            WQr = WQ.rearrange("(j p) n -> p j n", p=P)
            WKr = WK.rearrange("(j p) n -> p j n", p=P)
            WVr = WV.rearrange("(j p) n -> p j n", p=P)
            WOr = WO.rearrange("(j p) n -> p j n", p=P)
            WQs = persist.tile([P, 8, HG], bf16)
            WKs = persist.tile([P, 8, HG], bf16)
            for j in range(0, 8, 2):
                nc.gpsimd.dma_start(WQs[:, j : j + 2, :], WQr[:, j : j + 2, :])
                nc.gpsimd.dma_start(WKs[:, j : j + 2, :], WKr[:, j : j + 2, :])
            BQs = persist.tile([P, 2], f32)
            nc.gpsimd.dma_start(BQs, BQ.rearrange("(j p) -> p j", p=P))
            BKs = persist.tile([P, 2], f32)
            nc.gpsimd.dma_start(BKs, BK.rearrange("(j p) -> p j", p=P))

            # ---- X^T loaded directly (pre-transposed on host) as plain
            # contiguous DMAs, column-chunked so the first q-chunk of every
            # d-slab lands quickly and projections start early. ----
            XT = [
                persist.tile([P, S], bf16, name=f"XT{j}") for j in range(D // P)
            ]
            for nq in range(NQC):
                csl = slice(QC * nq, QC * (nq + 1))
                for j in range(D // P):
                    nc.sync.dma_start(
                        XT[j][:, csl], XTD[P * j : P * (j + 1), csl]
                    )
                if nq == 0:
                    WVs = persist.tile([P, 8, HG], bf16)
                    for j in range(0, 8, 2):
                        nc.gpsimd.dma_start(
                            WVs[:, j : j + 2, :], WVr[:, j : j + 2, :]
                        )
                    bv1 = persist.tile([1, HG], f32)
                    nc.gpsimd.dma_start(bv1, BV[None, :])
            bvb = persist.tile([P, HG], f32)
            nc.gpsimd.partition_broadcast(bvb, bv1[0:1, :])
            WOs = persist.tile([P, 2, D], bf16)
            for j in range(2):
                nc.gpsimd.dma_start(WOs[:, j, :], WOr[:, j, :])

            # ---- persistent activations ----
            QT = [persist.tile([P, S], bf16, name=f"QT{pp}") for pp in range(2)]
            KT = [persist.tile([P, S], bf16, name=f"KT{pp}") for pp in range(2)]
            # V4[p, t, h, d] = (X @ WV + BV)[128*t + p, 64*h + d]; d=64 -> 1.0
            V4 = persist.tile([P, NKB, H_PER_CORE, HD + 1], bf16)
            ones_f32 = persist.tile([P, NKB * H_PER_CORE], f32)
            nc.gpsimd.memset(ones_f32, 1.0)
            ones_bf = persist.tile([1, HD], bf16)
            nc.vector.tensor_copy(ones_bf, ones_f32[0:1, 0:HD])
            nc.vector.tensor_copy(
                V4[:, :, :, HD], ones_f32.rearrange("p (t h) -> p t h", t=NKB)
            )
            # ONT[p, j, q] = O_normalized[q, 128*j + p]
            ONT = persist.tile([P, 2, S], bf16)

            def emit_v_proj(t):
                psv = ps.tile([P, 512], f32, tag="proj", bufs=2, name=f"psv{t}")
                for j in range(8):
                    nc.tensor.matmul(
                        psv[:, :HG],
                        XT[j][:, P * t : P * (t + 1)],
                        WVs[:, j, :],
                        start=(j == 0),
                        stop=(j == 7),
                    )
                nc.vector.tensor_tensor(
                    out=V4[:, t, :, 0:HD],
                    in0=psv[:, :HG].rearrange("p (h d) -> p h d", h=H_PER_CORE),
                    in1=bvb.rearrange("p (h d) -> p h d", h=H_PER_CORE),
                    op=mybir.AluOpType.add,
                )

            def emit_q_chain(pp, nq):
                sl = slice(QC * nq, QC * (nq + 1))
                psq = ps.tile(
                    [P, 512], f32, tag="proj", bufs=2, name=f"psq{pp}_{nq}"
                )
                for j in range(8):
                    nc.tensor.matmul(
                        psq,
                        WQs[:, j, P * pp : P * (pp + 1)],
                        XT[j][:, sl],
                        start=(j == 0),
                        stop=(j == 7),
                    )
                nc.vector.tensor_scalar_add(QT[pp][:, sl], psq, BQs[:, pp : pp + 1])

            def emit_k_chain(pp, nq):
                sl = slice(QC * nq, QC * (nq + 1))
                psk = ps.tile(
                    [P, 512], f32, tag="proj", bufs=2, name=f"psk{pp}_{nq}"
                )
                for j in range(8):
                    nc.tensor.matmul(
                        psk,
                        WKs[:, j, P * pp : P * (pp + 1)],
                        XT[j][:, sl],
                        start=(j == 0),
                        stop=(j == 7),
                    )
                nc.vector.tensor_scalar_add(KT[pp][:, sl], psk, BKs[:, pp : pp + 1])

            def emit_qk_proj(pp, nq):
                emit_q_chain(pp, nq)
                emit_k_chain(pp, nq)

            def emit_pv(pp, qc, ot, kbs, pt):
                qb = QC * qc
                nkb = 4 * qc + 4
                for ii, kb in enumerate(kbs):
                    qloc = max(0, P * kb - qb)
                    for hh in range(2):
                        h = 2 * pp + hh
                        nc.tensor.matmul(
                            ot[hh][:, qloc:QC],
                            V4[:, kb, h, :],
                            pt[hh][:, ii, qloc:QC],
                            start=(kb == 0),
                            stop=(kb == nkb - 1),
                        )

            def emit_attention(pp, qc, fillers=(), fast_norm=False):
                fillers = list(fillers)

                def pop_filler():
                    if fillers:
                        fillers.pop(0)()

                qb = QC * qc
                qsl = slice(qb, qb + QC)
                nkb = 4 * qc + 4  # causal: k-blocks 0..nkb-1
                ot = [
                    ps.tile(
                        [HD + 1, QC], f32, tag=f"ot{hh}", bufs=1,
                        name=f"ot{hh}_{pp}_{qc}",
                    )
                    for hh in range(2)
                ]
                pending = []
                for kg in range((nkb + 1) // 2):
                    kbs = [kb for kb in (2 * kg, 2 * kg + 1) if kb < nkb]
                    st = [
                        ps.tile(
                            [P, 2, QC], f32, tag="sc", bufs=2,
                            name=f"sc{hh}_{pp}_{qc}_{kg}",
                        )
                        for hh in range(2)
                    ]
                    for ii, kb in enumerate(kbs):
                        # columns left of the diagonal block are fully masked:
                        # skip them in the matmul (exp still covers the full
                        # tile; the stale columns are never consumed).
                        qloc = max(0, P * kb - qb)
                        for hh in range(2):
                            hsl = slice(HD * hh, HD * (hh + 1))
                            nc.tensor.matmul(
                                st[hh][:, ii, qloc:],
                                KT[pp][hsl, P * kb : P * (kb + 1)],
                                QT[pp][hsl, qb + qloc : qb + QC],
                                start=True,
                                stop=True,
                            )
                    pt = [
                        sb.tile(
                            [P, 2, QC], bf16, tag=f"pt{hh}", bufs=4,
                            name=f"pt{hh}_{pp}_{qc}_{kg}",
                        )
                        for hh in range(2)
                    ]
                    for hh in range(2):
                        nexp = len(kbs)
                        nc.scalar.activation(
                            pt[hh][:, :nexp, :],
                            st[hh][:, :nexp, :],
                            mybir.ActivationFunctionType.Exp,
                            bias=0.0,
                            scale=0.125,
                        )
                    for ii, kb in enumerate(kbs):
                        if kb >= 4 * qc:  # diagonal block: causal zeroing
                            qloc = P * kb - qb
                            for hh in range(2):
                                blk = pt[hh][:, ii, qloc : qloc + P]
                                nc.gpsimd.affine_select(
                                    out=blk,
                                    in_=blk,
                                    compare_op=mybir.AluOpType.is_ge,
                                    fill=0.0,
                                    base=0,
                                    pattern=[[1, P]],  # iota = q' - k
                                    channel_multiplier=-1,
                                )
                    pop_filler()
                    pending.append((kbs, pt))
                    if len(pending) > 3:  # 3-group lookahead for the PE stream
                        emit_pv(pp, qc, ot, *pending.pop(0))
                        pop_filler()
                for item in pending:
                    emit_pv(pp, qc, ot, *item)
                    pop_filler()

                if fast_norm:
                    # last attention block: the normalize chain is on the
                    # critical path to the final output projection. Skip the
                    # SBUF evacuation, multiply straight from PSUM, and
                    # interleave the two heads' chains so DVE never stalls
                    # behind a GpSimd broadcast.
                    # l-row copies on ScalarE (free at the tail, PSUM-fast),
                    # reciprocals on DVE, broadcasts on GpSimd; the DMA-gated
                    # hh=1 product first.
                    ls2 = []
                    for hh in range(2):
                        l1 = sb.tile(
                            [1, QC], f32, tag="l", bufs=2,
                            name=f"l{hh}_{pp}_{qc}",
                        )
                        nc.scalar.copy(l1, ot[hh][HD : HD + 1, :])
                        nc.vector.reciprocal_approx_fast(l1, l1)
                        ls2.append(l1)
                    rs = []
                    for hh in range(2):
                        r = sb.tile(
                            [HD, QC], f32, tag="rr", bufs=2,
                            name=f"r{hh}_{pp}_{qc}",
                        )
                        nc.gpsimd.partition_broadcast(r, ls2[hh][0:1, :])
                        rs.append(r)
                    tmp = sb.tile(
                        [HD, QC], bf16, tag="nb", bufs=2, name=f"nb_{pp}_{qc}"
                    )
                    nc.vector.tensor_tensor(
                        out=tmp,
                        in0=ot[1][0:HD, :],
                        in1=rs[1],
                        op=mybir.AluOpType.mult,
                    )
                    for cc in range(4):
                        csl = slice(qb + 128 * cc, qb + 128 * (cc + 1))
                        nc.sync.dma_start(
                            ONT[HD:P, pp, csl], tmp[:, 128 * cc : 128 * (cc + 1)]
                        )
                    nc.vector.tensor_tensor(
                        out=ONT[0:HD, pp, qsl],
                        in0=ot[0][0:HD, :],
                        in1=rs[0],
                        op=mybir.AluOpType.mult,
                    )
                    while fillers:
                        fillers.pop(0)()
                    return

                # evacuate O^T (+ sums row) to SBUF right away to free the
                # PSUM accumulator banks, then normalize off-PSUM.
                oc = []
                ls = []
                for hh in range(2):
                    c = sb.tile(
                        [HD, QC], f32, tag="oc", bufs=2,
                        name=f"oc{hh}_{pp}_{qc}",
                    )
                    nc.vector.tensor_copy(c, ot[hh][0:HD, :])
                    l1 = sb.tile(
                        [1, QC], f32, tag="l", bufs=2, name=f"l{hh}_{pp}_{qc}"
                    )
                    nc.vector.tensor_copy(l1, ot[hh][HD : HD + 1, :])
                    oc.append(c)
                    ls.append(l1)
                for hh in range(2):
                    r = sb.tile(
                        [HD, QC], f32, tag="rr", bufs=2, name=f"r{hh}_{pp}_{qc}"
                    )
                    nc.vector.reciprocal_approx_fast(ls[hh], ls[hh])
                    nc.gpsimd.partition_broadcast(r, ls[hh][0:1, :])
                    if hh == 0:
                        nc.vector.tensor_tensor(
                            out=ONT[0:HD, pp, qsl],
                            in0=oc[hh][0:HD, :],
                            in1=r,
                            op=mybir.AluOpType.mult,
                        )
                    else:
                        tmp = sb.tile(
                            [HD, QC], bf16, tag="nb", bufs=2, name=f"nb_{pp}_{qc}"
                        )
                        nc.vector.tensor_tensor(
                            out=tmp,
                            in0=oc[hh][0:HD, :],
                            in1=r,
                            op=mybir.AluOpType.mult,
                        )
                        nc.sync.dma_start(ONT[HD:P, pp, qsl], tmp)
                while fillers:
                    fillers.pop(0)()

            def emit_oproj_t(t, use_sc=False):
                if use_sc:
                    # the attention score banks are free once the last
                    # exp has run: borrowing them doubles the number of
                    # output-projection chains in flight at the kernel tail.
                    ppair = ps.tile(
                        [P, 2, QC], f32, tag="sc", bufs=2, name=f"psy_sc{t}"
                    )
                    psys = [ppair[:, 0, :], ppair[:, 1, :]]
                else:
                    psys = [
                        ps.tile(
                            [P, 512], f32, tag="proj", bufs=2,
                            name=f"psy{t}_{nn}",
                        )
                        for nn in range(2)
                    ]
                for nn in range(2):
                    for j in range(2):
                        nc.tensor.matmul(
                            psys[nn],
                            ONT[:, j, P * t : P * (t + 1)],
                            WOs[:, j, 512 * nn : 512 * (nn + 1)],
                            start=(j == 0),
                            stop=(j == 1),
                        )
                for nn in range(2):
                    ysb = sb.tile(
                        [P, 512], bf16, tag="ysb", bufs=8,
                        name=f"ysb{t}_{nn}",
                    )
                    if use_sc and nn == 1:
                        # kernel tail: ScalarE is idle — split the PSUM
                        # evacuations across both engines.
                        nc.scalar.copy(ysb, psys[nn])
                    else:
                        nc.vector.tensor_copy(ysb, psys[nn])
                    for cc in range(2):
                        nc.sync.dma_start(
                            Y_pt[:, t, 512 * nn + 256 * cc :
                                 512 * nn + 256 * (cc + 1)],
                            ysb[:, 256 * cc : 256 * (cc + 1)],
                        )

            def emit_oproj(qc):
                for i, t in enumerate(range(4 * qc, 4 * qc + 4)):
                    emit_oproj_t(t, use_sc=(i % 2 == 1))

            # ---- interleaved schedule: filler matmul chains are threaded
            # between attention score groups so the PE never drains while
            # ScalarE chews through the exps. ----
            emit_qk_proj(0, 0)
            for t in range(2):
                emit_v_proj(t)
            for qc in range(NQC):
                f0 = []
                f1 = []
                if qc == 0:
                    f0.append(lambda: emit_v_proj(2))
                    f0.append(lambda: emit_v_proj(3))
                    f0.append(lambda: emit_q_chain(1, 0))
                    f0.append(lambda: emit_k_chain(1, 0))
                if qc < NQC - 1:
                    f0 += [
                        (lambda t=t: emit_v_proj(t))
                        for t in range(4 * qc + 4, 4 * qc + 8)
                    ]
                    f0.append(lambda nq=qc + 1: emit_q_chain(0, nq))
                    f0.append(lambda nq=qc + 1: emit_k_chain(0, nq))
                    f1.append(lambda nq=qc + 1: emit_q_chain(1, nq))
                    f1.append(lambda nq=qc + 1: emit_k_chain(1, nq))
                if qc > 0:
                    prev = [
                        (lambda t=t: emit_oproj_t(t))
                        for t in range(4 * (qc - 1), 4 * qc)
                    ]
                    if qc == NQC - 1:
                        # last q-chunk has no projection fillers left:
                        # split the previous chunk's output projection
                        # between both head-pair attention calls so the PE
                        # never drains while ScalarE chews the last exps.
                        f0 += prev[:2]
                        f1 += prev[2:]
                    else:
                        f1 += prev
                emit_attention(0, qc, f0)
                emit_attention(1, qc, f1, fast_norm=(qc == NQC - 1))
            emit_oproj(NQC - 1)

    nc.compile()
    return nc


_NC_CACHE = None


def _get_nc():
    global _NC_CACHE
    if _NC_CACHE is None:
        _NC_CACHE = build_nc()
    return _NC_CACHE


def _make_in_maps(inputs):
    bf = ml_dtypes.bfloat16
    X = np.asarray(inputs["X"], np.float32)
    Wq = np.asarray(inputs["Wq"], np.float32)
    Wk = np.asarray(inputs["Wk"], np.float32)
    Wv = np.asarray(inputs["Wv"], np.float32)
    Wo = np.asarray(inputs["Wo"], np.float32)
    bq = np.asarray(inputs["bq"], np.float32)
    bk = np.asarray(inputs["bk"], np.float32)
    bv = np.asarray(inputs["bv"], np.float32)
    in_maps = []
    for c in range(8):
        b, hg = c // 4, c % 4
        sl = slice(HG * hg, HG * (hg + 1))
        in_maps.append(
            {
                "XT": np.ascontiguousarray(X[b].T.astype(bf)),
                "WQ": np.ascontiguousarray(Wq[:, sl].astype(bf)),
                "WK": np.ascontiguousarray(Wk[:, sl].astype(bf)),
                "WV": np.ascontiguousarray(Wv[:, sl].astype(bf)),
                "WO": np.ascontiguousarray(Wo[sl, :].astype(bf)),
                "BQ": np.ascontiguousarray(bq[sl]),
                "BK": np.ascontiguousarray(bk[sl]),
                "BV": np.ascontiguousarray(bv[sl]),
            }
        )
    return in_maps


def kernel(X, Wq, bq, Wk, bk, Wv, bv, Wo, bo):
    X = np.asarray(X, np.float32)
    Wq = np.asarray(Wq, np.float32)
    Wk = np.asarray(Wk, np.float32)
    Wv = np.asarray(Wv, np.float32)
    Wo = np.asarray(Wo, np.float32)
    bq = np.asarray(bq, np.float32)
    bk = np.asarray(bk, np.float32)
    bv = np.asarray(bv, np.float32)
    bo = np.asarray(bo, np.float32)

    nc = _get_nc()
    in_maps = _make_in_maps(
        dict(X=X, Wq=Wq, bq=bq, Wk=Wk, bk=bk, Wv=Wv, bv=bv, Wo=Wo, bo=bo)
    )
    res = run_bass_kernel_spmd(nc, in_maps, core_ids=list(range(8)))
    ys = [np.asarray(r["Y"], np.float32) for r in res.results]
    out = np.stack(
        [ys[0] + ys[1] + ys[2] + ys[3], ys[4] + ys[5] + ys[6] + ys[7]]
    )
    return (out + bo).astype(np.float32)



# revision 41
# speedup vs baseline: 1.0219x; 1.0219x over previous
"""Causal multi-head attention on 8 TRN2 NeuronCores.

Sharding: data-parallel over batch (2) x tensor-parallel over heads (4 groups
of 4 heads). Core c handles batch c//4, heads [4*(c%4), 4*(c%4)+4).
Each core computes Q/K/V projections for its head slice, causal flash-style
attention, and a partial output projection (Wo row-shard). The host sums the
4 partials per batch and adds bo.

Matmuls run in bf16 (fp32 accumulation in PSUM); X and the weight slices are
pre-cast to bf16 on the host so X^T comes from XBAR DMA transposes instead of
PE transposes. Softmax runs unnormalized (scores are ~N(0,1), no max
subtraction needed); the per-row sums ride along as a 65th column of V.

Shapes (per core): X [2048, 1024], WQ/WK/WV [1024, 256], WO [256, 1024].
"""

import ml_dtypes
import numpy as np

import concourse.bass as bass
import concourse.mybir as mybir
import concourse.tile as tile
from concourse import bacc
from concourse.bass_utils import run_bass_kernel_spmd

B = 2
S = 2048
D = 1024
H_PER_CORE = 4  # heads per core
HD = 64  # head dim
HG = H_PER_CORE * HD  # 256: projection slice width per core
P = 128
NQC = 4  # q chunks of 512
QC = S // NQC  # 512
NKB = S // P  # 16 k-blocks of 128
NEG = -1.0e9

f32 = mybir.dt.float32
bf16 = mybir.dt.bfloat16


def build_nc():
    nc = bacc.Bacc()

    XTD = nc.dram_tensor("XT", [D, S], bf16, kind="ExternalInput")
    WQ = nc.dram_tensor("WQ", [D, HG], bf16, kind="ExternalInput")
    WK = nc.dram_tensor("WK", [D, HG], bf16, kind="ExternalInput")
    WV = nc.dram_tensor("WV", [D, HG], bf16, kind="ExternalInput")
    WO = nc.dram_tensor("WO", [HG, D], bf16, kind="ExternalInput")
    BQ = nc.dram_tensor("BQ", [HG], f32, kind="ExternalInput")
    BK = nc.dram_tensor("BK", [HG], f32, kind="ExternalInput")
    BV = nc.dram_tensor("BV", [HG], f32, kind="ExternalInput")
    Y = nc.dram_tensor("Y", [S, D], bf16, kind="ExternalOutput")

    Y_pt = Y.rearrange("(t p) d -> p t d", p=P)

    with tile.TileContext(nc) as tc:
        with (
            tc.tile_pool(name="persist", bufs=1) as persist,
            tc.tile_pool(name="sb", bufs=2) as sb,
            tc.tile_pool(name="ps", bufs=1, space="PSUM") as ps,
        ):
            # ---- input DMAs. dma_start costs ~0.6-0.8us of ISSUE time on
            # the issuing engine, so spread issuance across engines that are
            # idle at the head (sync/vector/scalar/gpsimd) and keep the
            # number of DMAs small. ----
            WQr = WQ.rearrange("(j p) n -> p j n", p=P)
            WKr = WK.rearrange("(j p) n -> p j n", p=P)
            WVr = WV.rearrange("(j p) n -> p j n", p=P)
            WOr = WO.rearrange("(j p) n -> p j n", p=P)
            WQs = persist.tile([P, 8, HG], bf16)
            WKs = persist.tile([P, 8, HG], bf16)
            XT = [
                persist.tile([P, S], bf16, name=f"XT{j}") for j in range(D // P)
            ]
            c0 = slice(0, QC)
            c1 = slice(QC, 2 * QC)
            c23 = slice(2 * QC, S)
            # sync: X^T slabs 0-3 chunk0/1, then the tail halves
            # scalar: X^T slabs 4-7 chunk0/1, WK, BK (its exps start ~18us)
            # gpsimd: WQ, BQ, WV, bv, WO
            for j in range(4):
                nc.sync.dma_start(XT[j][:, c0], XTD[P * j : P * (j + 1), c0])
                nc.scalar.dma_start(
                    XT[j + 4][:, c0], XTD[P * (j + 4) : P * (j + 5), c0]
                )
            for j in range(0, 8, 4):
                nc.gpsimd.dma_start(WQs[:, j : j + 4, :], WQr[:, j : j + 4, :])
                nc.scalar.dma_start(WKs[:, j : j + 4, :], WKr[:, j : j + 4, :])
            BQs = persist.tile([P, 2], f32)
            nc.gpsimd.dma_start(BQs, BQ.rearrange("(j p) -> p j", p=P))
            BKs = persist.tile([P, 2], f32)
            nc.scalar.dma_start(BKs, BK.rearrange("(j p) -> p j", p=P))
            for j in range(4):
                nc.sync.dma_start(XT[j][:, c1], XTD[P * j : P * (j + 1), c1])
                nc.scalar.dma_start(
                    XT[j + 4][:, c1], XTD[P * (j + 4) : P * (j + 5), c1]
                )
            WVs = persist.tile([P, 8, HG], bf16)
            for j in range(0, 8, 4):
                nc.gpsimd.dma_start(WVs[:, j : j + 4, :], WVr[:, j : j + 4, :])
            bv1 = persist.tile([1, HG], f32)
            nc.gpsimd.dma_start(bv1, BV[None, :])
            for j in range(8):
                nc.sync.dma_start(
                    XT[j][:, c23], XTD[P * j : P * (j + 1), c23]
                )
            bvb = persist.tile([P, HG], f32)
            nc.gpsimd.partition_broadcast(bvb, bv1[0:1, :])
            WOs = persist.tile([P, 2, D], bf16)
            for j in range(2):
                nc.gpsimd.dma_start(WOs[:, j, :], WOr[:, j, :])

            # ---- persistent activations ----
            QT = [persist.tile([P, S], bf16, name=f"QT{pp}") for pp in range(2)]
            KT = [persist.tile([P, S], bf16, name=f"KT{pp}") for pp in range(2)]
            # V4[p, t, h, d] = (X @ WV + BV)[128*t + p, 64*h + d]; d=64 -> 1.0
            V4 = persist.tile([P, NKB, H_PER_CORE, HD + 1], bf16)
            ones_f32 = persist.tile([P, NKB * H_PER_CORE], f32)
            nc.gpsimd.memset(ones_f32, 1.0)
            ones_bf = persist.tile([1, HD], bf16)
            nc.vector.tensor_copy(ones_bf, ones_f32[0:1, 0:HD])
            nc.vector.tensor_copy(
                V4[:, :, :, HD], ones_f32.rearrange("p (t h) -> p t h", t=NKB)
            )
            # ONT[p, j, q] = O_normalized[q, 128*j + p]
            ONT = persist.tile([P, 2, S], bf16)

            def emit_v_proj(t):
                psv = ps.tile([P, 512], f32, tag="proj", bufs=2, name=f"psv{t}")
                for j in range(8):
                    nc.tensor.matmul(
                        psv[:, :HG],
                        XT[j][:, P * t : P * (t + 1)],
                        WVs[:, j, :],
                        start=(j == 0),
                        stop=(j == 7),
                    )
                nc.vector.tensor_tensor(
                    out=V4[:, t, :, 0:HD],
                    in0=psv[:, :HG].rearrange("p (h d) -> p h d", h=H_PER_CORE),
                    in1=bvb.rearrange("p (h d) -> p h d", h=H_PER_CORE),
                    op=mybir.AluOpType.add,
                )

            def emit_q_chain(pp, nq):
                sl = slice(QC * nq, QC * (nq + 1))
                psq = ps.tile(
                    [P, 512], f32, tag="proj", bufs=2, name=f"psq{pp}_{nq}"
                )
                for j in range(8):
                    nc.tensor.matmul(
                        psq,
                        WQs[:, j, P * pp : P * (pp + 1)],
                        XT[j][:, sl],
                        start=(j == 0),
                        stop=(j == 7),
                    )
                nc.vector.tensor_scalar_add(QT[pp][:, sl], psq, BQs[:, pp : pp + 1])

            def emit_k_chain(pp, nq):
                sl = slice(QC * nq, QC * (nq + 1))
                psk = ps.tile(
                    [P, 512], f32, tag="proj", bufs=2, name=f"psk{pp}_{nq}"
                )
                for j in range(8):
                    nc.tensor.matmul(
                        psk,
                        WKs[:, j, P * pp : P * (pp + 1)],
                        XT[j][:, sl],
                        start=(j == 0),
                        stop=(j == 7),
                    )
                nc.vector.tensor_scalar_add(KT[pp][:, sl], psk, BKs[:, pp : pp + 1])

            def emit_qk_proj(pp, nq):
                emit_q_chain(pp, nq)
                emit_k_chain(pp, nq)

            def emit_pv(pp, qc, ot, kbs, pt):
                qb = QC * qc
                nkb = 4 * qc + 4
                for ii, kb in enumerate(kbs):
                    qloc = max(0, P * kb - qb)
                    for hh in range(2):
                        h = 2 * pp + hh
                        nc.tensor.matmul(
                            ot[hh][:, qloc:QC],
                            V4[:, kb, h, :],
                            pt[hh][:, ii, qloc:QC],
                            start=(kb == 0),
                            stop=(kb == nkb - 1),
                        )

            def emit_attention(pp, qc, fillers=(), fast_norm=False):
                fillers = list(fillers)

                def pop_filler():
                    if fillers:
                        fillers.pop(0)()

                qb = QC * qc
                qsl = slice(qb, qb + QC)
                nkb = 4 * qc + 4  # causal: k-blocks 0..nkb-1
                ot = [
                    ps.tile(
                        [HD + 1, QC], f32, tag=f"ot{hh}", bufs=1,
                        name=f"ot{hh}_{pp}_{qc}",
                    )
                    for hh in range(2)
                ]
                pending = []
                for kg in range((nkb + 1) // 2):
                    kbs = [kb for kb in (2 * kg, 2 * kg + 1) if kb < nkb]
                    st = [
                        ps.tile(
                            [P, 2, QC], f32, tag="sc", bufs=2,
                            name=f"sc{hh}_{pp}_{qc}_{kg}",
                        )
                        for hh in range(2)
                    ]
                    for ii, kb in enumerate(kbs):
                        # columns left of the diagonal block are fully masked:
                        # skip them in the matmul (exp still covers the full
                        # tile; the stale columns are never consumed).
                        qloc = max(0, P * kb - qb)
                        for hh in range(2):
                            hsl = slice(HD * hh, HD * (hh + 1))
                            nc.tensor.matmul(
                                st[hh][:, ii, qloc:],
                                KT[pp][hsl, P * kb : P * (kb + 1)],
                                QT[pp][hsl, qb + qloc : qb + QC],
                                start=True,
                                stop=True,
                            )
                    pt = [
                        sb.tile(
                            [P, 2, QC], bf16, tag=f"pt{hh}", bufs=4,
                            name=f"pt{hh}_{pp}_{qc}_{kg}",
                        )
                        for hh in range(2)
                    ]
                    for hh in range(2):
                        nexp = len(kbs)
                        nc.scalar.activation(
                            pt[hh][:, :nexp, :],
                            st[hh][:, :nexp, :],
                            mybir.ActivationFunctionType.Exp,
                            bias=0.0,
                            scale=0.125,
                        )
                    for ii, kb in enumerate(kbs):
                        if kb >= 4 * qc:  # diagonal block: causal zeroing
                            qloc = P * kb - qb
                            for hh in range(2):
                                blk = pt[hh][:, ii, qloc : qloc + P]
                                nc.gpsimd.affine_select(
                                    out=blk,
                                    in_=blk,
                                    compare_op=mybir.AluOpType.is_ge,
                                    fill=0.0,
                                    base=0,
                                    pattern=[[1, P]],  # iota = q' - k
                                    channel_multiplier=-1,
                                )
                    pop_filler()
                    pending.append((kbs, pt))
                    if len(pending) > 3:  # 3-group lookahead for the PE stream
                        emit_pv(pp, qc, ot, *pending.pop(0))
                        pop_filler()
                for item in pending:
                    emit_pv(pp, qc, ot, *item)
                    pop_filler()

                if fast_norm:
                    # last attention block: the normalize chain is on the
                    # critical path to the final output projection. Skip the
                    # SBUF evacuation, multiply straight from PSUM, and
                    # interleave the two heads' chains so DVE never stalls
                    # behind a GpSimd broadcast.
                    # l-row copies on ScalarE (free at the tail, PSUM-fast),
                    # reciprocals on DVE, broadcasts on GpSimd; the DMA-gated
                    # hh=1 product first.
                    ls2 = []
                    for hh in range(2):
                        l1 = sb.tile(
                            [1, QC], f32, tag="l", bufs=2,
                            name=f"l{hh}_{pp}_{qc}",
                        )
                        nc.scalar.copy(l1, ot[hh][HD : HD + 1, :])
                        nc.vector.reciprocal_approx_fast(l1, l1)
                        ls2.append(l1)
                    rs = []
                    for hh in range(2):
                        r = sb.tile(
                            [HD, QC], f32, tag="rr", bufs=2,
                            name=f"r{hh}_{pp}_{qc}",
                        )
                        nc.gpsimd.partition_broadcast(r, ls2[hh][0:1, :])
                        rs.append(r)
                    tmp = sb.tile(
                        [HD, QC], bf16, tag="nb", bufs=2, name=f"nb_{pp}_{qc}"
                    )
                    nc.vector.tensor_tensor(
                        out=tmp,
                        in0=ot[1][0:HD, :],
                        in1=rs[1],
                        op=mybir.AluOpType.mult,
                    )
                    for cc in range(4):
                        csl = slice(qb + 128 * cc, qb + 128 * (cc + 1))
                        nc.sync.dma_start(
                            ONT[HD:P, pp, csl], tmp[:, 128 * cc : 128 * (cc + 1)]
                        )
                    nc.vector.tensor_tensor(
                        out=ONT[0:HD, pp, qsl],
                        in0=ot[0][0:HD, :],
                        in1=rs[0],
                        op=mybir.AluOpType.mult,
                    )
                    while fillers:
                        fillers.pop(0)()
                    return

                # evacuate O^T (+ sums row) to SBUF right away to free the
                # PSUM accumulator banks, then normalize off-PSUM.
                oc = []
                ls = []
                for hh in range(2):
                    c = sb.tile(
                        [HD, QC], f32, tag="oc", bufs=2,
                        name=f"oc{hh}_{pp}_{qc}",
                    )
                    nc.vector.tensor_copy(c, ot[hh][0:HD, :])
                    l1 = sb.tile(
                        [1, QC], f32, tag="l", bufs=2, name=f"l{hh}_{pp}_{qc}"
                    )
                    nc.vector.tensor_copy(l1, ot[hh][HD : HD + 1, :])
                    oc.append(c)
                    ls.append(l1)
                for hh in range(2):
                    r = sb.tile(
                        [HD, QC], f32, tag="rr", bufs=2, name=f"r{hh}_{pp}_{qc}"
                    )
                    nc.vector.reciprocal_approx_fast(ls[hh], ls[hh])
                    nc.gpsimd.partition_broadcast(r, ls[hh][0:1, :])
                    if hh == 0:
                        nc.vector.tensor_tensor(
                            out=ONT[0:HD, pp, qsl],
                            in0=oc[hh][0:HD, :],
                            in1=r,
                            op=mybir.AluOpType.mult,
                        )
                    else:
                        tmp = sb.tile(
                            [HD, QC], bf16, tag="nb", bufs=2, name=f"nb_{pp}_{qc}"
                        )
                        nc.vector.tensor_tensor(
                            out=tmp,
                            in0=oc[hh][0:HD, :],
                            in1=r,
                            op=mybir.AluOpType.mult,
                        )
                        nc.sync.dma_start(ONT[HD:P, pp, qsl], tmp)
                while fillers:
                    fillers.pop(0)()

            def emit_oproj_t(t, use_sc=False):
                if use_sc:
                    # the attention score banks are free once the last
                    # exp has run: borrowing them doubles the number of
                    # output-projection chains in flight at the kernel tail.
                    ppair = ps.tile(
                        [P, 2, QC], f32, tag="sc", bufs=2, name=f"psy_sc{t}"
                    )
                    psys = [ppair[:, 0, :], ppair[:, 1, :]]
                else:
                    psys = [
                        ps.tile(
                            [P, 512], f32, tag="proj", bufs=2,
                            name=f"psy{t}_{nn}",
                        )
                        for nn in range(2)
                    ]
                for nn in range(2):
                    for j in range(2):
                        nc.tensor.matmul(
                            psys[nn],
                            ONT[:, j, P * t : P * (t + 1)],
                            WOs[:, j, 512 * nn : 512 * (nn + 1)],
                            start=(j == 0),
                            stop=(j == 1),
                        )
                for nn in range(2):
                    ysb = sb.tile(
                        [P, 512], bf16, tag="ysb", bufs=8,
                        name=f"ysb{t}_{nn}",
                    )
                    if use_sc and nn == 1:
                        # kernel tail: ScalarE is idle — split the PSUM
                        # evacuations across both engines.
                        nc.scalar.copy(ysb, psys[nn])
                    else:
                        nc.vector.tensor_copy(ysb, psys[nn])
                    if use_sc:
                        # tail: two chunks on two queues to cut drain time
                        for cc in range(2):
                            nc.sync.dma_start(
                                Y_pt[:, t, 512 * nn + 256 * cc :
                                     512 * nn + 256 * (cc + 1)],
                                ysb[:, 256 * cc : 256 * (cc + 1)],
                            )
                    else:
                        nc.sync.dma_start(
                            Y_pt[:, t, 512 * nn : 512 * (nn + 1)], ysb
                        )

            def emit_oproj(qc):
                for i, t in enumerate(range(4 * qc, 4 * qc + 4)):
                    emit_oproj_t(t, use_sc=(i % 2 == 1))

            # ---- interleaved schedule: filler matmul chains are threaded
            # between attention score groups so the PE never drains while
            # ScalarE chews through the exps. ----
            emit_qk_proj(0, 0)
            for t in range(2):
                emit_v_proj(t)
            for qc in range(NQC):
                f0 = []
                f1 = []
                if qc == 0:
                    f0.append(lambda: emit_v_proj(2))
                    f0.append(lambda: emit_v_proj(3))
                    f0.append(lambda: emit_q_chain(1, 0))
                    f0.append(lambda: emit_k_chain(1, 0))
                if qc < NQC - 1:
                    f0 += [
                        (lambda t=t: emit_v_proj(t))
                        for t in range(4 * qc + 4, 4 * qc + 8)
                    ]
                    f0.append(lambda nq=qc + 1: emit_q_chain(0, nq))
                    f0.append(lambda nq=qc + 1: emit_k_chain(0, nq))
                    f1.append(lambda nq=qc + 1: emit_q_chain(1, nq))
                    f1.append(lambda nq=qc + 1: emit_k_chain(1, nq))
                if qc > 0:
                    prev = [
                        (lambda t=t: emit_oproj_t(t))
                        for t in range(4 * (qc - 1), 4 * qc)
                    ]
                    if qc == NQC - 1:
                        # last q-chunk has no projection fillers left:
                        # split the previous chunk's output projection
                        # between both head-pair attention calls so the PE
                        # never drains while ScalarE chews the last exps.
                        f0 += prev[:2]
                        f1 += prev[2:]
                    else:
                        f1 += prev
                emit_attention(0, qc, f0)
                emit_attention(1, qc, f1, fast_norm=(qc == NQC - 1))
            emit_oproj(NQC - 1)

    nc.compile()
    return nc


_NC_CACHE = None


def _get_nc():
    global _NC_CACHE
    if _NC_CACHE is None:
        _NC_CACHE = build_nc()
    return _NC_CACHE


def _make_in_maps(inputs):
    bf = ml_dtypes.bfloat16
    X = np.asarray(inputs["X"], np.float32)
    Wq = np.asarray(inputs["Wq"], np.float32)
    Wk = np.asarray(inputs["Wk"], np.float32)
    Wv = np.asarray(inputs["Wv"], np.float32)
    Wo = np.asarray(inputs["Wo"], np.float32)
    bq = np.asarray(inputs["bq"], np.float32)
    bk = np.asarray(inputs["bk"], np.float32)
    bv = np.asarray(inputs["bv"], np.float32)
    in_maps = []
    for c in range(8):
        b, hg = c // 4, c % 4
        sl = slice(HG * hg, HG * (hg + 1))
        in_maps.append(
            {
                "XT": np.ascontiguousarray(X[b].T.astype(bf)),
                "WQ": np.ascontiguousarray(Wq[:, sl].astype(bf)),
                "WK": np.ascontiguousarray(Wk[:, sl].astype(bf)),
                "WV": np.ascontiguousarray(Wv[:, sl].astype(bf)),
                "WO": np.ascontiguousarray(Wo[sl, :].astype(bf)),
                "BQ": np.ascontiguousarray(bq[sl]),
                "BK": np.ascontiguousarray(bk[sl]),
                "BV": np.ascontiguousarray(bv[sl]),
            }
        )
    return in_maps


def kernel(X, Wq, bq, Wk, bk, Wv, bv, Wo, bo):
    X = np.asarray(X, np.float32)
    Wq = np.asarray(Wq, np.float32)
    Wk = np.asarray(Wk, np.float32)
    Wv = np.asarray(Wv, np.float32)
    Wo = np.asarray(Wo, np.float32)
    bq = np.asarray(bq, np.float32)
    bk = np.asarray(bk, np.float32)
    bv = np.asarray(bv, np.float32)
    bo = np.asarray(bo, np.float32)

    nc = _get_nc()
    in_maps = _make_in_maps(
        dict(X=X, Wq=Wq, bq=bq, Wk=Wk, bk=bk, Wv=Wv, bv=bv, Wo=Wo, bo=bo)
    )
    res = run_bass_kernel_spmd(nc, in_maps, core_ids=list(range(8)))
    ys = [np.asarray(r["Y"], np.float32) for r in res.results]
    out = np.stack(
        [ys[0] + ys[1] + ys[2] + ys[3], ys[4] + ys[5] + ys[6] + ys[7]]
    )
    return (out + bo).astype(np.float32)

